# revision 45
# baseline (speedup 1.0000x reference)
"""Trainium2 Bass kernel for 3-level inverse Haar DWT (DWTInverse, db1).

Strategy (data-parallel): batch N=16 is sharded 2-per-core across 8 NeuronCores.
Per (sample) the 2D inverse DWT at each level is a pointwise 4->4 butterfly
over (ll, lh, hl, hh) producing a 2x2 output block:
    E[2j]   = ll + s2*(lh+hl+hh)     (even row)
    E[2j+1] = ll + s2*(lh-hl-hh)
    O[2j]   = ll - s2*(lh-hl+hh)  ...  (odd row)
with the per-level scale folded into scalar_tensor_tensor ops (out = in0*sigma op in1).

Layout trick: row-parity "phase planes".  Level-k ll is stored as F=2^k SBUF
planes of 128 partitions; plane f, partition i holds image row F*i+f.  The
column interleave is done with stride-2 writes in the free dim (free on DVE f32
1x mode); the row interleave is never materialized on-chip - the next level
loads its yh rows with a matching stride-F DMA access pattern, and the final
level stores each plane with a stride-8 row pattern directly into the output.
Channels ride along in the free dimension (3x wider tiles, fewer ops).
"""

import sys

if "/opt/trn_rl_repo" not in sys.path:
    sys.path.insert(0, "/opt/trn_rl_repo")

import numpy as np

import concourse.bass as bass
import concourse.mybir as mybir
from concourse.tile import TileContext
from concourse import bass_utils

F32 = mybir.dt.float32
N_CORES = 8


def _split_excess_waits(nc, maxw=1):
    """Post-pass over the assembled BIR: this walrus build rejects
    instructions carrying more than a couple of semaphore waits ("Too many
    sync wait commands").  For any instruction with more than `maxw` waits,
    move the excess onto single-wait carrier NOPs inserted just before it on
    the same engine - the engine's instruction stream stalls at each carrier
    until its condition holds, which is semantically identical to one
    instruction waiting on all of them.
    """
    k = 0
    for fn in nc.m.functions:
        for bb in fn.blocks:
            new = []
            changed = False
            for inst in bb.instructions:
                si = inst.sync_info
                if si is not None and len(si.on_wait) > maxw:
                    waits = list(si.on_wait)
                    excess, keep = waits[:-maxw], waits[-maxw:]
                    for i in range(0, len(excess), maxw):
                        k += 1
                        new.append(
                            mybir.InstNoOp(
                                name=f"waitsplit_{k}",
                                engine=inst.engine,
                                bass_nofuse=True,
                                sync_info=mybir.SyncInfo(
                                    on_wait=excess[i:i + maxw], on_update=[]
                                ),
                            )
                        )
                    inst.sync_info = mybir.SyncInfo(
                        on_wait=keep, on_update=list(si.on_update)
                    )
                    changed = True
                new.append(inst)
            if changed:
                bb.instructions = new
    return nc


def _emit_level(nc, pool, npart, in_planes, yh_dram, W, sigma, h_bufs,
                out_sink, d_eng, fine_tail=False):
    """One DWT synthesis level for one sample.

    in_planes: list of F tiles [128, 3, W]; plane f partition i = ll row F*i+f.
    yh_dram:   AP (3, 3, F*128, W) = (c, s, r, j) for this sample.
    out_sink(two_f, plane): receives output plane [128, 3, 2W] for phase 2f(+1).

    d_eng: engine for the d = hl-hh op.  Measured: keeping everything on DVE
    wins - concurrent GpSimd elementwise degrades DVE throughput (SBUF port
    sharing), and PE identity-matmul offload loses to LDWEIGHTS + cold-HAM
    fp32 rates.  DVE runs all 8 butterfly ops (f32 is 1x mode regardless, so
    the strided interleaving writes are free).
    """
    F = len(in_planes)
    mult, add = mybir.AluOpType.mult, mybir.AluOpType.add
    stt = nc.vector.scalar_tensor_tensor

    def _load_strips():
        if F == 4:
            # L0 only: pair adjacent phase strips per DMA (rows 4i+2g, 4i+2g+1
            # are adjacent in DRAM -> 4 KB per-partition chunks instead of
            # 2 KB, ~6% more DMA bandwidth where the kernel is BW-tightest),
            # split by subband so each strip's ops wait only on 1/3 of it
            yh_rs = yh_dram.rearrange("c s (i g p) j -> s i g c p j", g=2, p=2)
            for g in range(2):
                bands = []
                for si, bn in enumerate(("lh", "hl", "hh")):
                    # each slot holds a PAIR of strips -> half the buf count
                    t = pool.tile([npart, 3, 2, W], F32, tag=f"{bn}{W}",
                                  bufs=max(1, h_bufs // 2), name=f"{bn}{W}_{g}")
                    nc.sync.dma_start(out=t[:], in_=yh_rs[si, :, g])
                    bands.append(t)
                for p in range(2):
                    f = g * 2 + p
                    yield (f, bands[0][:, :, p, :], bands[1][:, :, p, :],
                           bands[2][:, :, p, :])
        else:
            yh_r = yh_dram.rearrange("c s (i f) j -> i f c s j", f=F)
            for f in range(F):
                # channel-major free layout so the DRAM-side (c, s) dims merge
                # into one AP dim (DMA supports at most 3 dims after balancing)
                h = pool.tile([npart, 3, 3, W], F32, tag=f"h{W}", bufs=h_bufs,
                              name=f"h{W}_{f}")
                nc.sync.dma_start(out=h[:], in_=yh_r[:, f])
                yield f, h[:, :, 0, :], h[:, :, 1, :], h[:, :, 2, :]

    for f, lh, hl, hh in _load_strips():
        P = in_planes[f]
        a = pool.tile([npart, 3, W], F32, tag="a")
        b = pool.tile([npart, 3, W], F32, tag="b")
        c = pool.tile([npart, 3, W], F32, tag="c")
        d = pool.tile([npart, 3, W], F32, tag="d")
        nc.vector.tensor_add(out=c[:], in0=hl, in1=hh)
        d_eng.tensor_sub(out=d[:], in0=hl, in1=hh)
        stt(out=a[:], in0=lh, scalar=sigma, in1=P[:], op0=mult, op1=add)
        stt(out=b[:], in0=lh, scalar=-sigma, in1=P[:], op0=mult, op1=add)
        for phi, (lo, hi) in enumerate(((a, c), (b, d))):
            plane = out_sink.alloc(2 * f + phi)
            if fine_tail and f == F - 1:
                # final strip of the kernel: compute + store per channel so
                # the store drain overlaps the last compute ops and the final
                # store is 1/3 size - trims the end-of-kernel DMA tail
                for ch in range(3):
                    stt(out=plane[:, ch, 0::2], in0=hi[:, ch], scalar=sigma,
                        in1=lo[:, ch], op0=mult, op1=add)
                    stt(out=plane[:, ch, 1::2], in0=hi[:, ch], scalar=-sigma,
                        in1=lo[:, ch], op0=mult, op1=add)
                    out_sink.emit_ch(2 * f + phi, plane, ch)
            else:
                stt(out=plane[:, :, 0::2], in0=hi[:], scalar=sigma, in1=lo[:],
                    op0=mult, op1=add)
                stt(out=plane[:, :, 1::2], in0=hi[:], scalar=-sigma, in1=lo[:],
                    op0=mult, op1=add)
                out_sink.emit(2 * f + phi, plane)


class _PlaneSink:
    """Collects intermediate level output planes in SBUF."""

    def __init__(self, pool, npart, W, tag, bufs):
        self.pool, self.npart, self.W, self.tag, self.bufs = pool, npart, W, tag, bufs
        self.planes = [None] * 100

    def alloc(self, two_f):
        return self.pool.tile([self.npart, 3, 2 * self.W], F32, tag=self.tag,
                              bufs=self.bufs, name=f"{self.tag}_{two_f}")

    def emit(self, two_f, plane):
        self.planes[two_f] = plane


class _DramSink:
    """Streams final level output planes straight to the DRAM output.

    (Measured: merging the E/O planes of a strip into one 8 KB-chunk store
    cuts DMA-engine busy ~155us -> ~136us, but the later store start and
    coarser staging stall DVE more than the DMA saving - separate per-plane
    4 KB-row stores are faster end-to-end.)
    """

    def __init__(self, nc, pool, npart, y_dram_n, W, bufs):
        self.nc, self.pool, self.npart, self.W, self.bufs = nc, pool, npart, W, bufs
        # (c, r, j) with r = 8*i + f  ->  (i, f, c, j)
        self.y_r = y_dram_n.rearrange("c (i f) j -> i f c j", f=8)

    def alloc(self, two_f):
        return self.pool.tile([self.npart, 3, 2 * self.W], F32, tag="eo",
                              bufs=self.bufs, name=f"eo_{two_f}")

    def _eng(self, two_f):
        # alternate planes across both HWDGE queues (ACT + SP): stores drain
        # at double bandwidth and a compute-waiting store never blocks more
        # than one queue's loads
        return self.nc.scalar if two_f % 2 == 0 else self.nc.sync

    def emit(self, two_f, plane):
        self._eng(two_f).dma_start(out=self.y_r[:, two_f], in_=plane[:])

    def emit_ch(self, two_f, plane, ch):
        self._eng(two_f + ch).dma_start(out=self.y_r[:, two_f, ch],
                                        in_=plane[:, ch])


def build_nc(ns, s, h0=128, bufs_cfg=None):
    """Build the Bass module for `ns` samples per core, base size h0 (=128)."""
    cfg = dict(h2=2, h1=2, h0=4, p2=2, q=4, eo=3, d_pool=False)
    # per-partition SBUF: lh/hl/hh512 3x2x12K=72K + h256 18K + h128 9K + yl 6K + abcd 24K
    # + p2 12K + q 24K + eo 36K ~= 201K of 207.9K
    if bufs_cfg:
        cfg.update(bufs_cfg)
    s2 = s * s
    h1, h2_, h3 = h0, 2 * h0, 4 * h0  # yh2/yh1/yh0 spatial sizes
    nc = bass.Bass()
    yl = nc.dram_tensor("yl", (ns, 3, h0, h0), F32, kind="ExternalInput")
    yh2 = nc.dram_tensor("yh2", (ns, 3, 3, h1, h1), F32, kind="ExternalInput")
    yh1 = nc.dram_tensor("yh1", (ns, 3, 3, h2_, h2_), F32, kind="ExternalInput")
    yh0 = nc.dram_tensor("yh0", (ns, 3, 3, h3, h3), F32, kind="ExternalInput")
    y = nc.dram_tensor("y", (ns, 3, 8 * h0, 8 * h0), F32, kind="ExternalOutput")
    npart = h0  # partitions actually used (h0 for full problem = 128)

    with TileContext(nc) as tc:
        with tc.tile_pool(name="p", bufs=1) as pool:
            d_eng = nc.gpsimd if cfg["d_pool"] else nc.vector
            for n in range(ns):
                # ---- level 2 input: yl scaled by s^6 ----
                ylt = pool.tile([npart, 3, h0], F32, tag="ylt", bufs=2)
                nc.sync.dma_start(out=ylt[:], in_=yl[n].rearrange("c r j -> r c j"))
                yls = pool.tile([npart, 3, h0], F32, tag="yls", bufs=2)
                nc.scalar.mul(out=yls[:], in_=ylt[:], mul=s2 ** 3)

                # ---- level 2: 128 -> 256 ----
                sink2 = _PlaneSink(pool, npart, h0, "p2", cfg["p2"] * 2)
                _emit_level(nc, pool, npart, [yls], yh2[n], h0, s2 ** 3,
                            cfg["h2"], sink2, d_eng)

                # ---- level 1: 256 -> 512 ----
                sink1 = _PlaneSink(pool, npart, 2 * h0, "q", cfg["q"])
                _emit_level(nc, pool, npart, sink2.planes[:2], yh1[n], 2 * h0,
                            s2 ** 2, cfg["h1"], sink1, d_eng)

                # ---- level 0: 512 -> 1024, streamed to DRAM ----
                sink0 = _DramSink(nc, pool, npart, y[n], 4 * h0, cfg["eo"])
                _emit_level(nc, pool, npart, sink1.planes[:4], yh0[n], 4 * h0,
                            s2, cfg["h0"], sink0, d_eng,
                            fine_tail=(n == ns - 1))
    return _split_excess_waits(nc)


_CACHE = {}


def kernel(yl, yh0, yh1, yh2, g0, g1):
    yl = np.ascontiguousarray(yl, dtype=np.float32)
    yh0 = np.ascontiguousarray(yh0, dtype=np.float32)
    yh1 = np.ascontiguousarray(yh1, dtype=np.float32)
    yh2 = np.ascontiguousarray(yh2, dtype=np.float32)
    s = float(np.asarray(g0)[0])
    assert np.allclose(np.asarray(g0), [s, s], atol=1e-6)
    assert np.allclose(np.asarray(g1), [s, -s], atol=1e-6)

    n = yl.shape[0]
    assert n % N_CORES == 0
    ns = n // N_CORES

    key = (ns, round(s, 12))
    if key not in _CACHE:
        _CACHE[key] = build_nc(ns, s)
    nc = _CACHE[key]

    in_maps = [
        {
            "yl": yl[i * ns:(i + 1) * ns],
            "yh0": yh0[i * ns:(i + 1) * ns],
            "yh1": yh1[i * ns:(i + 1) * ns],
            "yh2": yh2[i * ns:(i + 1) * ns],
        }
        for i in range(N_CORES)
    ]
    res = bass_utils.run_bass_kernel_spmd(nc, in_maps, core_ids=list(range(N_CORES)))
    return np.concatenate([res.results[i]["y"] for i in range(N_CORES)], axis=0)


# revision 46
# speedup vs baseline: 1.1040x; 1.1040x over previous
"""Trainium2 Bass kernel for 3-level inverse Haar DWT (DWTInverse, db1).

Strategy (data-parallel): batch N=16 is sharded 2-per-core across 8 NeuronCores.
Per (sample) the 2D inverse DWT at each level is a pointwise 4->4 butterfly
over (ll, lh, hl, hh) producing a 2x2 output block:
    E[2j]   = ll + s2*(lh+hl+hh)     (even row)
    E[2j+1] = ll + s2*(lh-hl-hh)
    O[2j]   = ll - s2*(lh-hl+hh)  ...  (odd row)
with the per-level scale folded into scalar_tensor_tensor ops (out = in0*sigma op in1).

Layout trick: row-parity "phase planes".  Level-k ll is stored as F=2^k SBUF
planes of 128 partitions; plane f, partition i holds image row F*i+f.  The
column interleave is done with stride-2 writes in the free dim (free on DVE f32
1x mode); the row interleave is never materialized on-chip - the next level
loads its yh rows with a matching stride-F DMA access pattern, and the final
level stores each plane with a stride-8 row pattern directly into the output.
Channels ride along in the free dimension (3x wider tiles, fewer ops).
"""

import sys

if "/opt/trn_rl_repo" not in sys.path:
    sys.path.insert(0, "/opt/trn_rl_repo")

import numpy as np

import concourse.bass as bass
import concourse.mybir as mybir
from concourse.tile import TileContext
from concourse import bass_utils

F32 = mybir.dt.float32
N_CORES = 8


def _split_excess_waits(nc, maxw=1):
    """Post-pass over the assembled BIR: this walrus build rejects
    instructions carrying more than a couple of semaphore waits ("Too many
    sync wait commands").  For any instruction with more than `maxw` waits,
    move the excess onto single-wait carrier NOPs inserted just before it on
    the same engine - the engine's instruction stream stalls at each carrier
    until its condition holds, which is semantically identical to one
    instruction waiting on all of them.
    """
    k = 0
    for fn in nc.m.functions:
        for bb in fn.blocks:
            new = []
            changed = False
            for inst in bb.instructions:
                si = inst.sync_info
                if si is not None and len(si.on_wait) > maxw:
                    waits = list(si.on_wait)
                    excess, keep = waits[:-maxw], waits[-maxw:]
                    for i in range(0, len(excess), maxw):
                        k += 1
                        new.append(
                            mybir.InstNoOp(
                                name=f"waitsplit_{k}",
                                engine=inst.engine,
                                bass_nofuse=True,
                                sync_info=mybir.SyncInfo(
                                    on_wait=excess[i:i + maxw], on_update=[]
                                ),
                            )
                        )
                    inst.sync_info = mybir.SyncInfo(
                        on_wait=keep, on_update=list(si.on_update)
                    )
                    changed = True
                new.append(inst)
            if changed:
                bb.instructions = new
    return nc


def _emit_level(nc, pool, npart, in_planes, yh_dram, W, sigma, h_bufs,
                out_sink, d_eng, fine_tail=False):
    """One DWT synthesis level for one sample.

    in_planes: list of F tiles [128, 3, W]; plane f partition i = ll row F*i+f.
    yh_dram:   AP (3, 3, F*128, W) = (c, s, r, j) for this sample.
    out_sink(two_f, plane): receives output plane [128, 3, 2W] for phase 2f(+1).

    d_eng: engine for the d = hl-hh op.  Measured: keeping everything on DVE
    wins - concurrent GpSimd elementwise degrades DVE throughput (SBUF port
    sharing), and PE identity-matmul offload loses to LDWEIGHTS + cold-HAM
    fp32 rates.  DVE runs all 8 butterfly ops (f32 is 1x mode regardless, so
    the strided interleaving writes are free).
    """
    F = len(in_planes)
    mult, add = mybir.AluOpType.mult, mybir.AluOpType.add
    stt = nc.vector.scalar_tensor_tensor
    yh_r = yh_dram.rearrange("c s (i f) j -> i f c s j", f=F)
    for f, P in enumerate(in_planes):
        # channel-major free layout so the DRAM-side (c, s) dims merge into
        # one AP dim (DMA supports at most 3 dims after balancing)
        h = pool.tile([npart, 3, 3, W], F32, tag=f"h{W}", bufs=h_bufs)
        nc.sync.dma_start(out=h[:], in_=yh_r[:, f])
        lh, hl, hh = h[:, :, 0, :], h[:, :, 1, :], h[:, :, 2, :]
        a = pool.tile([npart, 3, W], F32, tag="a")
        b = pool.tile([npart, 3, W], F32, tag="b")
        c = pool.tile([npart, 3, W], F32, tag="c")
        d = pool.tile([npart, 3, W], F32, tag="d")
        nc.vector.tensor_add(out=c[:], in0=hl, in1=hh)
        d_eng.tensor_sub(out=d[:], in0=hl, in1=hh)
        stt(out=a[:], in0=lh, scalar=sigma, in1=P[:], op0=mult, op1=add)
        stt(out=b[:], in0=lh, scalar=-sigma, in1=P[:], op0=mult, op1=add)
        for phi, (lo, hi) in enumerate(((a, c), (b, d))):
            plane = out_sink.alloc(2 * f + phi)
            if fine_tail and f == F - 1:
                # final strip of the kernel: compute + store per channel so
                # the store drain overlaps the last compute ops and the final
                # store is 1/3 size - trims the end-of-kernel DMA tail
                for ch in range(3):
                    stt(out=plane[:, ch, 0::2], in0=hi[:, ch], scalar=sigma,
                        in1=lo[:, ch], op0=mult, op1=add)
                    stt(out=plane[:, ch, 1::2], in0=hi[:, ch], scalar=-sigma,
                        in1=lo[:, ch], op0=mult, op1=add)
                    out_sink.emit_ch(2 * f + phi, plane, ch)
            else:
                stt(out=plane[:, :, 0::2], in0=hi[:], scalar=sigma, in1=lo[:],
                    op0=mult, op1=add)
                stt(out=plane[:, :, 1::2], in0=hi[:], scalar=-sigma, in1=lo[:],
                    op0=mult, op1=add)
                out_sink.emit(2 * f + phi, plane)


class _PlaneSink:
    """Collects intermediate level output planes in SBUF."""

    def __init__(self, pool, npart, W, tag, bufs):
        self.pool, self.npart, self.W, self.tag, self.bufs = pool, npart, W, tag, bufs
        self.planes = [None] * 100

    def alloc(self, two_f):
        return self.pool.tile([self.npart, 3, 2 * self.W], F32, tag=self.tag,
                              bufs=self.bufs, name=f"{self.tag}_{two_f}")

    def emit(self, two_f, plane):
        self.planes[two_f] = plane


class _DramSink:
    """Streams final level output planes straight to the DRAM output.

    (Measured: merging the E/O planes of a strip into one 8 KB-chunk store
    cuts DMA-engine busy ~155us -> ~136us, but the later store start and
    coarser staging stall DVE more than the DMA saving - separate per-plane
    4 KB-row stores are faster end-to-end.)
    """

    def __init__(self, nc, pool, npart, y_dram_n, W, bufs):
        self.nc, self.pool, self.npart, self.W, self.bufs = nc, pool, npart, W, bufs
        # (c, r, j) with r = 8*i + f  ->  (i, f, c, j)
        self.y_r = y_dram_n.rearrange("c (i f) j -> i f c j", f=8)

    def alloc(self, two_f):
        return self.pool.tile([self.npart, 3, 2 * self.W], F32, tag="eo",
                              bufs=self.bufs, name=f"eo_{two_f}")

    def _eng(self, two_f):
        # alternate planes across both HWDGE queues (ACT + SP): stores drain
        # at double bandwidth and a compute-waiting store never blocks more
        # than one queue's loads
        return self.nc.scalar if two_f % 2 == 0 else self.nc.sync

    def emit(self, two_f, plane):
        self._eng(two_f).dma_start(out=self.y_r[:, two_f], in_=plane[:])

    def emit_ch(self, two_f, plane, ch):
        self._eng(two_f + ch).dma_start(out=self.y_r[:, two_f, ch],
                                        in_=plane[:, ch])


def build_nc(ns, s, h0=128, bufs_cfg=None):
    """Build the Bass module for `ns` samples per core, base size h0 (=128)."""
    cfg = dict(h2=2, h1=2, h0=4, p2=2, q=4, eo=3, d_pool=False)
    # per-partition SBUF: h512 72K + h256 18K + h128 9K + yl 6K + abcd 24K
    # + p2 12K + q 24K + eo 36K ~= 201K of 207.9K
    if bufs_cfg:
        cfg.update(bufs_cfg)
    s2 = s * s
    h1, h2_, h3 = h0, 2 * h0, 4 * h0  # yh2/yh1/yh0 spatial sizes
    nc = bass.Bass()
    yl = nc.dram_tensor("yl", (ns, 3, h0, h0), F32, kind="ExternalInput")
    yh2 = nc.dram_tensor("yh2", (ns, 3, 3, h1, h1), F32, kind="ExternalInput")
    yh1 = nc.dram_tensor("yh1", (ns, 3, 3, h2_, h2_), F32, kind="ExternalInput")
    yh0 = nc.dram_tensor("yh0", (ns, 3, 3, h3, h3), F32, kind="ExternalInput")
    y = nc.dram_tensor("y", (ns, 3, 8 * h0, 8 * h0), F32, kind="ExternalOutput")
    npart = h0  # partitions actually used (h0 for full problem = 128)

    with TileContext(nc) as tc:
        with tc.tile_pool(name="p", bufs=1) as pool:
            d_eng = nc.gpsimd if cfg["d_pool"] else nc.vector
            for n in range(ns):
                # ---- level 2 input: yl scaled by s^6 ----
                ylt = pool.tile([npart, 3, h0], F32, tag="ylt", bufs=2)
                nc.sync.dma_start(out=ylt[:], in_=yl[n].rearrange("c r j -> r c j"))
                yls = pool.tile([npart, 3, h0], F32, tag="yls", bufs=2)
                nc.scalar.mul(out=yls[:], in_=ylt[:], mul=s2 ** 3)

                # ---- level 2: 128 -> 256 ----
                sink2 = _PlaneSink(pool, npart, h0, "p2", cfg["p2"] * 2)
                _emit_level(nc, pool, npart, [yls], yh2[n], h0, s2 ** 3,
                            cfg["h2"], sink2, d_eng)

                # ---- level 1: 256 -> 512 ----
                sink1 = _PlaneSink(pool, npart, 2 * h0, "q", cfg["q"])
                _emit_level(nc, pool, npart, sink2.planes[:2], yh1[n], 2 * h0,
                            s2 ** 2, cfg["h1"], sink1, d_eng)

                # ---- level 0: 512 -> 1024, streamed to DRAM ----
                sink0 = _DramSink(nc, pool, npart, y[n], 4 * h0, cfg["eo"])
                _emit_level(nc, pool, npart, sink1.planes[:4], yh0[n], 4 * h0,
                            s2, cfg["h0"], sink0, d_eng,
                            fine_tail=(n == ns - 1))
    return _split_excess_waits(nc)


_CACHE = {}


def kernel(yl, yh0, yh1, yh2, g0, g1):
    yl = np.ascontiguousarray(yl, dtype=np.float32)
    yh0 = np.ascontiguousarray(yh0, dtype=np.float32)
    yh1 = np.ascontiguousarray(yh1, dtype=np.float32)
    yh2 = np.ascontiguousarray(yh2, dtype=np.float32)
    s = float(np.asarray(g0)[0])
    assert np.allclose(np.asarray(g0), [s, s], atol=1e-6)
    assert np.allclose(np.asarray(g1), [s, -s], atol=1e-6)

    n = yl.shape[0]
    assert n % N_CORES == 0
    ns = n // N_CORES

    key = (ns, round(s, 12))
    if key not in _CACHE:
        _CACHE[key] = build_nc(ns, s)
    nc = _CACHE[key]

    in_maps = [
        {
            "yl": yl[i * ns:(i + 1) * ns],
            "yh0": yh0[i * ns:(i + 1) * ns],
            "yh1": yh1[i * ns:(i + 1) * ns],
            "yh2": yh2[i * ns:(i + 1) * ns],
        }
        for i in range(N_CORES)
    ]
    res = bass_utils.run_bass_kernel_spmd(nc, in_maps, core_ids=list(range(N_CORES)))
    return np.concatenate([res.results[i]["y"] for i in range(N_CORES)], axis=0)
